# revision 1
# baseline (speedup 1.0000x reference)
"""Multi-head attention (B=2, N=2048, C=1024, H=16, qk-RMSNorm) on 8 TRN2 cores.

Restructured v2 of the baseline kernel. Same sharding (TP over 4 head
groups x DP over batch; host sums the 4 w_proj partials per batch).

Scheduling changes vs baseline:
- x arrives in j-block slices (rotating tiles), v/qk-gen/rmsnorm are
  interleaved per block, so the PE starts ~2us in and never stalls on
  DMA (the baseline lost ~60us to startup + stalls + pstate resets).
- Attention pipeline is 2 deep: iteration idx runs S(cur) || AV(prev),
  then norm(prev2) + proj, so the softmax-denominator chain never
  blocks the PE in program order.
- Projection is w-stationary with transposed output [C, n]: bias folds
  into the per-partition DVE merge (saves 16k bias-matmul columns);
  host transposes.
- RMSNorm of block j is deferred into block j+1's matmul stream (and
  the last block's into the attention boundary), so its DVE chain never
  parks at the head of the PE queue.
- q/k/pt/v are bf16 (same PE rate as fp32r, half SBUF/DMA); the qkv/
  proj GEMMs stay fp32r.

PSUM budget (8 banks x 2KB/partition): tag "s2" [128,1024] bufs=2 (S
slots, qk/v accs, proj accs, denominator broadcasts), tag "oas"
[128,512] bufs=4 (AV accumulators + rmsnorm sumsq).
"""

import sys

if "/opt/trn_rl_repo" not in sys.path:
    sys.path.insert(0, "/opt/trn_rl_repo")

from contextlib import ExitStack

import numpy as np

import concourse.mybir as mybir
import concourse.tile as tile
from concourse import bacc
from concourse.bass_utils import run_bass_kernel_spmd

F32 = mybir.dt.float32
F32R = mybir.dt.float32r
BF16 = mybir.dt.bfloat16
AF = mybir.ActivationFunctionType

B, N, C, H = 2, 2048, 1024, 16
D = C // H          # 64
EPS = 1e-6
NCORES = 8
GROUPS = 4          # head groups (cores per batch)
HL = H // GROUPS    # heads per core = 4
CL = HL * D         # local channels = 256
SCALE = D ** -0.5   # 0.125

P = 128             # partition dim
KT = C // P         # 8 contraction tiles over C
NQ = 512            # query/token block
HPB = P // D        # heads per 128-channel block = 2
VW = D + 1          # 65: v columns + ones column


def build(n=N, nq=NQ):
    nb = n // P          # key blocks of 128
    nj = n // nq         # token blocks of nq
    kt = KT

    nc = bacc.Bacc("TRN2", target_bir_lowering=False, debug=False,
                   num_devices=NCORES)

    xT_d = nc.dram_tensor("xT", [C, n], F32, kind="ExternalInput").ap()
    wqk_d = nc.dram_tensor("w_qk", [C, 2 * CL], F32, kind="ExternalInput").ap()
    wv_d = nc.dram_tensor("w_v", [C, CL], F32, kind="ExternalInput").ap()
    wpr_d = nc.dram_tensor("w_pr", [CL, C], F32, kind="ExternalInput").ap()
    bqk_d = nc.dram_tensor("b_qk", [P, 4], F32, kind="ExternalInput").ap()
    bv_d = nc.dram_tensor("b_v", [1, CL], F32, kind="ExternalInput").ap()
    bprT_d = nc.dram_tensor("b_prT", [P, C // P], F32, kind="ExternalInput").ap()
    qkw_d = nc.dram_tensor("qkw", [P, 4], F32, kind="ExternalInput").ap()
    outT_d = nc.dram_tensor("outT", [C, n], F32, kind="ExternalOutput").ap()

    with tile.TileContext(nc) as tc, ExitStack() as ctx:
        con = ctx.enter_context(tc.tile_pool(name="con", bufs=1))
        wp = ctx.enter_context(tc.tile_pool(name="wp", bufs=1))
        qk = ctx.enter_context(tc.tile_pool(name="qk", bufs=1))
        vp = ctx.enter_context(tc.tile_pool(name="vp", bufs=1))
        xp = ctx.enter_context(tc.tile_pool(name="xp", bufs=1))
        sqp = ctx.enter_context(tc.tile_pool(name="sqp", bufs=4))
        rp = ctx.enter_context(tc.tile_pool(name="rp", bufs=2))
        ptp = ctx.enter_context(tc.tile_pool(name="ptp", bufs=1))
        atp = ctx.enter_context(tc.tile_pool(name="atp", bufs=1))
        rp2 = ctx.enter_context(tc.tile_pool(name="rp2", bufs=2))
        osp = ctx.enter_context(tc.tile_pool(name="osp", bufs=4))
        ps = ctx.enter_context(tc.tile_pool(name="ps", bufs=1, space="PSUM"))

        bv_sb = con.tile([1, CL], F32R, tag="bv")
        bqk_sb = con.tile([P, 4], F32, tag="bqk")
        qkw_sb = con.tile([P, 4], F32, tag="qkw")
        bprT_sb = con.tile([P, C // P], F32, tag="bprT")

        # ---- constants (compute engines; no DMA) ----
        ones_f = con.tile([P, P], F32, tag="onesf")
        nc.vector.memset(ones_f[:], 1.0)
        ones_r = con.tile([1, P], F32R, tag="onesr")     # lhsT for v bias bcast
        nc.vector.tensor_copy(ones_r[:], ones_f[0:1, :])
        ones_m = con.tile([P, P], BF16, tag="onesm")     # lhsT for sumsq bcast
        nc.vector.memset(ones_m[:], 1.0)
        eps_sb = con.tile([P, 1], F32, tag="eps")
        nc.vector.memset(eps_sb[:], EPS)
        # ---- weight tiles (DMAs interleaved with x below) ----
        wv_sb = [wp.tile([P, CL], F32R, tag=f"wv{k}", name=f"wv{k}") for k in range(kt)]
        wqk_sb = [wp.tile([P, 2 * CL], F32R, tag=f"wqk{k}", name=f"wqk{k}") for k in range(kt)]
        wpr_sb = [wp.tile([P, C], F32R, tag=f"wpr{t}", name=f"wpr{t}") for t in range(CL // P)]

        # rotating x tiles: 2 j-blocks in flight per k
        def x_tile(j, k):
            return xp.tile([P, nq], F32R, tag=f"xt{k}", bufs=2, name=f"xt{k}_{j}")

        # persistent attention operands
        qkT = [qk.tile([P, n], BF16, tag=f"qkT{m}", name=f"qkT{m}") for m in range(4)]
        v_aug = [vp.tile([P, HL * VW], BF16, tag=f"va{i}", name=f"va{i}") for i in range(nb)]
        attnT = [atp.tile([P, n], F32R, tag=f"at{t}", name=f"at{t}") for t in range(HL // HPB)]

        # startup: v weights + x(j=0) interleaved, consts slotted in after
        # the first pair, then qk/proj weights.
        xs = {}
        for k in range(kt):
            nc.sync.dma_start(wv_sb[k][:], wv_d[k * P:(k + 1) * P, :].bitcast(F32R))
            t = x_tile(0, k)
            nc.sync.dma_start(t[:], xT_d[k * P:(k + 1) * P, 0:nq].bitcast(F32R))
            xs[(0, k)] = t
            if k == 0:
                nc.sync.dma_start(bv_sb[:], bv_d[:].bitcast(F32R))
        nc.sync.dma_start(bqk_sb[:], bqk_d[:])
        nc.sync.dma_start(qkw_sb[:], qkw_d[:])
        for k in range(kt):
            nc.sync.dma_start(wqk_sb[k][:], wqk_d[k * P:(k + 1) * P, :].bitcast(F32R))
        nc.sync.dma_start(bprT_sb[:], bprT_d[:])
        for t in range(CL // P):
            nc.sync.dma_start(wpr_sb[t][:], wpr_d[t * P:(t + 1) * P, :].bitcast(F32R))

        # ---- stage 1+2, pipelined per token block j ----
        for j in range(nj):
            js = slice(j * nq, (j + 1) * nq)
            if j + 1 < nj:
                for k in range(kt):
                    t = x_tile(j + 1, k)
                    nc.sync.dma_start(
                        t[:], xT_d[k * P:(k + 1) * P,
                                   (j + 1) * nq:(j + 2) * nq].bitcast(F32R))
                    xs[(j + 1, k)] = t

            # v for this block's nq//P key blocks: natural layout + ones col
            for i in range(j * (nq // P), (j + 1) * (nq // P)):
                ioff = i * P - j * nq
                acc = ps.tile([P, CL], F32, tag="s2", bufs=2, name="vacc")
                for k in range(kt):
                    nc.tensor.matmul(
                        acc[:], xs[(j, k)][:, ioff:ioff + P], wv_sb[k][:],
                        start=(k == 0), stop=False)
                nc.tensor.matmul(acc[:], ones_r[0:1, 0:P], bv_sb[:],
                                 start=False, stop=True)
                for h in range(HL):
                    base = h * VW
                    nc.scalar.copy(
                        v_aug[i][:, base:base + D], acc[:, h * D:(h + 1) * D])
                    nc.gpsimd.tensor_copy(
                        v_aug[i][:, base + D:base + VW], ones_f[:, 0:1])

            # qk projection + rmsnorm, k-heads (m=2,3) first: the norm of
            # one pair overlaps the next pair's matmuls, and attention's kT
            # dependency clears while the q half of the last block computes
            def emit_qk(m):
                acc = ps.tile([P, nq], F32, tag="s2", bufs=2, name="qacc")
                for k in range(kt):
                    nc.tensor.matmul(
                        acc[:], wqk_sb[k][:, m * P:(m + 1) * P],
                        xs[(j, k)][:, :], start=(k == 0), stop=(k == kt - 1))
                nc.vector.tensor_scalar_add(
                    qkT[m][:, js], acc[:], bqk_sb[:, m:m + 1])

            def emit_rms_for(jr):
                jrs = slice(jr * nq, (jr + 1) * nq)

                def emit_rms(m):
                    sq = sqp.tile([P, nq], BF16, tag="sq", name="sq")
                    nc.vector.tensor_mul(sq[:], qkT[m][:, jrs], qkT[m][:, jrs])
                    for h2 in range(HPB):
                        pr = slice(h2 * D, (h2 + 1) * D)
                        ssq = ps.tile([P, nq], F32, tag="oas", bufs=4, name="ssq")
                        nc.tensor.matmul(ssq[:], ones_m[pr, :], sq[pr, :],
                                         start=True, stop=True)
                        rms = rp.tile([P, nq], F32, tag="rms", bufs=4, name="rms")
                        nc.scalar.activation(rms[:], ssq[:], AF.Sqrt,
                                             scale=1.0 / D, bias=eps_sb[:, 0:1])
                        rec = rp.tile([P, nq], F32, tag="rec", bufs=4, name="rec")
                        nc.vector.reciprocal_approx_fast(rec[:], rms[:])
                        nc.vector.scalar_tensor_tensor(
                            qkT[m][pr, jrs], qkT[m][pr, jrs], qkw_sb[pr, m:m + 1],
                            rec[pr, :], op0=mybir.AluOpType.mult,
                            op1=mybir.AluOpType.mult)

                return emit_rms

            for m in (2, 3, 0, 1):
                emit_qk(m)
            if j > 0:
                for m in (2, 3, 0, 1):
                    emit_rms_for(j - 1)(m)

        for m in (2, 3, 0, 1):
            emit_rms_for(nj - 1)(m)

        # ---- attention: S(cur) || AV(prev), then norm(prev2) + proj ----
        units = [(j, hp) for j in range(nj) for hp in range(HL // HPB)]
        BLK = min(4, nb)

        def emit_s(u, i):
            j, hp = u
            js = slice(j * nq, (j + 1) * nq)
            qm, km = hp, 2 + hp
            s2 = ps.tile([P, 2 * nq], F32, tag="s2", bufs=2, name="s2")
            for sub in range(HPB):
                pr = slice(sub * D, (sub + 1) * D)
                nc.tensor.matmul(
                    s2[:, sub * nq:(sub + 1) * nq],
                    qkT[km][pr, i * P:(i + 1) * P], qkT[qm][pr, js],
                    start=True, stop=True)
            pt = ptp.tile([P, 2 * nq], BF16, tag="pt", bufs=20, name="pt")
            nc.scalar.activation(pt[:], s2[:], AF.Exp, scale=SCALE)
            return pt

        def emit_av(u, oas, pts, i):
            j, hp = u
            for sub in range(HPB):
                h = hp * HPB + sub
                nc.tensor.matmul(
                    oas[sub][0:VW, :], v_aug[i][:, h * VW:(h + 1) * VW],
                    pts[i][:, sub * nq:(sub + 1) * nq],
                    start=(i == 0), stop=(i == nb - 1))

        def emit_norm(u, oas):
            # Softmax denominators ride in row 64 of each AV output. Copy
            # them to 1-partition rows, PE-broadcast each to 64 partitions
            # (both broadcasts share one PSUM slot; base-0 outputs only —
            # nonzero output tile positions with K=1 fail the ISA check),
            # reciprocal into SBUF, then normalize into attnT.
            j, hp = u
            js = slice(j * nq, (j + 1) * nq)
            for sub in range(HPB):
                sums = rp2.tile([1, nq], F32R, tag=f"sums{sub}", name="sums")
                nc.vector.tensor_copy(sums[:], oas[sub][D:VW, :])
                bcs = ps.tile([D, nq], F32, tag="s2", bufs=2, name="bcs")
                nc.tensor.matmul(bcs[:], ones_r[0:1, 0:D], sums[:],
                                 start=True, stop=True)
                recn = rp2.tile([D, nq], F32, tag=f"recn{sub}", name="recn")
                nc.vector.reciprocal_approx_fast(recn[:], bcs[:])
                pr = slice(sub * D, (sub + 1) * D)
                nc.vector.tensor_mul(attnT[hp][pr, js], oas[sub][0:D, :],
                                     recn[:])

        def emit_proj(j):
            js = slice(j * nq, (j + 1) * nq)
            for m in range(C // P):
                acc = ps.tile([P, nq], F32, tag="s2", bufs=2, name="pacc")
                for t in range(CL // P):
                    nc.tensor.matmul(
                        acc[:], wpr_sb[t][:, m * P:(m + 1) * P],
                        attnT[t][:, js], start=(t == 0), stop=(t == CL // P - 1))
                ost = osp.tile([P, nq], F32, tag="ost", name="ost")
                nc.vector.tensor_scalar_add(ost[:], acc[:], bprT_sb[:, m:m + 1])
                nc.sync.dma_start(outT_d[m * P:(m + 1) * P, js], ost[:])

        prev = None    # (unit, oas, pts)
        prev2 = None
        for idx in range(len(units) + 2):
            cur = units[idx] if idx < len(units) else None
            # norm/proj of prev2 first: their reads gate the slot reuse of
            # this iteration's AV writes, so they must precede them in
            # scheduler priority (emitting them later deadlocks the
            # in-order PE queue)
            if prev2 is not None:
                emit_norm(prev2[0], prev2[1])
                j2, hp2 = prev2[0]
                if hp2 == HL // HPB - 1:
                    emit_proj(j2)
            oas_prev = None
            if prev is not None:
                oas_prev = [ps.tile([P, nq], F32, tag="oas", bufs=4,
                                    name=f"oa{s_}") for s_ in range(HPB)]
            pts = {}
            for ib in range((nb + BLK - 1) // BLK):
                blk = range(ib * BLK, min((ib + 1) * BLK, nb))
                if cur is not None:
                    for i in blk:
                        pts[i] = emit_s(cur, i)
                if prev is not None:
                    for i in blk:
                        emit_av(prev[0], oas_prev, prev[2], i)
            prev2 = (prev[0], oas_prev) if prev is not None else None
            prev = (cur, None, pts) if cur is not None else None

    nc.compile()
    return nc


_NC_CACHE = {}


def _get_nc(n=N, nq=NQ):
    key = (n, nq)
    if key not in _NC_CACHE:
        _NC_CACHE[key] = build(n, nq)
    return _NC_CACHE[key]


def make_in_maps(x, w_qkv, b_qkv, q_w, k_w, w_proj, b_proj):
    """Shard full inputs into per-core in_maps (host side)."""
    in_maps = []
    for cid in range(NCORES):
        b, g = cid // GROUPS, cid % GROUPS
        c0 = g * CL
        xT = np.ascontiguousarray(x[b].T)
        w_qk = np.ascontiguousarray(
            np.concatenate([w_qkv[:, c0:c0 + CL],
                            w_qkv[:, C + c0:C + c0 + CL]], axis=1))
        w_v = np.ascontiguousarray(w_qkv[:, 2 * C + c0:2 * C + c0 + CL])
        w_pr = np.ascontiguousarray(w_proj[c0:c0 + CL, :])
        b_qk = np.stack([b_qkv[c0 + m * P:c0 + (m + 1) * P] for m in range(2)]
                        + [b_qkv[C + c0 + m * P:C + c0 + (m + 1) * P]
                           for m in range(2)], axis=1)
        b_v = b_qkv[2 * C + c0:2 * C + c0 + CL].reshape(1, CL)
        # host gather sums GROUPS partials per batch; split the bias so it
        # lands exactly once
        b_prT = np.ascontiguousarray((b_proj / GROUPS).reshape(C // P, P).T)
        qkw = np.stack([np.tile(q_w, HPB), np.tile(q_w, HPB),
                        np.tile(k_w, HPB), np.tile(k_w, HPB)], axis=1)
        in_maps.append({
            "xT": xT.astype(np.float32),
            "w_qk": w_qk.astype(np.float32),
            "w_v": w_v.astype(np.float32),
            "w_pr": w_pr.astype(np.float32),
            "b_qk": np.ascontiguousarray(b_qk).astype(np.float32),
            "b_v": b_v.astype(np.float32),
            "b_prT": b_prT.astype(np.float32),
            "qkw": np.ascontiguousarray(qkw).astype(np.float32),
        })
    return in_maps


def kernel(x, w_qkv, b_qkv, q_w, k_w, w_proj, b_proj, _trace=False):
    x = np.asarray(x)
    n = x.shape[1]
    nc = _get_nc(n, NQ if n % NQ == 0 else P)
    in_maps = make_in_maps(np.asarray(x, np.float32), np.asarray(w_qkv, np.float32),
                           np.asarray(b_qkv, np.float32), np.asarray(q_w, np.float32),
                           np.asarray(k_w, np.float32), np.asarray(w_proj, np.float32),
                           np.asarray(b_proj, np.float32))
    res = run_bass_kernel_spmd(nc, in_maps, core_ids=list(range(NCORES)),
                               trace=_trace)
    # TP unshard: sum the 4 head-group partials per batch, transpose, stack
    out = np.stack([
        sum(res.results[b * GROUPS + g]["outT"] for g in range(GROUPS)).T
        for b in range(B)
    ]).astype(np.float32)
    if _trace:
        return out, res
    return out



# revision 2
# speedup vs baseline: 1.1014x; 1.1014x over previous
"""Multi-head attention (B=2, N=2048, C=1024, H=16, qk-RMSNorm) on 8 TRN2 cores.

v3 of the kernel. Same sharding as baseline (TP over 4 head groups x DP
over batch; host sums the 4 w_proj partials per batch) and the same
attention pipeline (S(cur) || AV(prev), then norm(prev2) + proj).

Changes vs v2 (299.98us):
- fp16 everywhere 16-bit data goes: x, w_qkv, w_proj, qkT, pt, v_aug,
  attnT, sums. Same PE rate as fp32r/bf16, half the DMA + LDWEIGHTS
  bytes of the fp32r GEMM weights, and 4x less quantization noise than
  bf16 (fp16 has 10 mantissa bits vs bf16's 8; all magnitudes here are
  well inside fp16 range: |logits*scale| < ~6 -> pt < ~450 < 65504).
- v_aug is produced directly by the v GEMM: w_v is host-padded to
  [C, HL*65] with zero columns at the per-head ones positions and the
  bias row carries 1.0 there, so acc = x@wv_aug + ones*bv_aug lands in
  the augmented layout. One DVE copy per key block replaces the 4
  scalar.copy + 4 gpsimd ones-writes of v2 (~30us of ACT/Pool work).
- RMSNorm: the two heads of an m-slice share one ssq PSUM tile (the
  second head's sumsq matmul is tile-positioned at partition 64), so
  sqrt / reciprocal / scale-mul run once per m-slice on [128, nq]
  instead of per head on [64, nq]: halves those ACT/DVE instr counts.
  sq is computed from the fp16 qkT in SBUF (2x DVE mode) instead of
  from PSUM.

PSUM budget unchanged: tag "s2" [128,1024] bufs=2, tag "oas" [128,512]
bufs=4.
"""

import sys

if "/opt/trn_rl_repo" not in sys.path:
    sys.path.insert(0, "/opt/trn_rl_repo")

from contextlib import ExitStack

import numpy as np

import concourse.mybir as mybir
import concourse.tile as tile
from concourse import bacc
from concourse.bass_utils import run_bass_kernel_spmd

F32 = mybir.dt.float32
F16 = mybir.dt.float16
AF = mybir.ActivationFunctionType

B, N, C, H = 2, 2048, 1024, 16
D = C // H          # 64
EPS = 1e-6
NCORES = 8
GROUPS = 4          # head groups (cores per batch)
HL = H // GROUPS    # heads per core = 4
CL = HL * D         # local channels = 256
SCALE = D ** -0.5   # 0.125

P = 128             # partition dim
KT = C // P         # 8 contraction tiles over C
NQ = 512            # query/token block
HPB = P // D        # heads per 128-channel block = 2
VW = D + 1          # 65: v columns + ones column
CLA = HL * VW       # 260: augmented v width


def build(n=N, nq=NQ):
    nb = n // P          # key blocks of 128
    nj = n // nq         # token blocks of nq
    kt = KT

    nc = bacc.Bacc("TRN2", target_bir_lowering=False, debug=False,
                   num_devices=NCORES)

    xT_d = nc.dram_tensor("xT", [C, n], F16, kind="ExternalInput").ap()
    wqk_d = nc.dram_tensor("w_qk", [C, 2 * CL], F16, kind="ExternalInput").ap()
    wv_d = nc.dram_tensor("w_v", [C, CLA], F16, kind="ExternalInput").ap()
    wpr_d = nc.dram_tensor("w_pr", [CL, C], F16, kind="ExternalInput").ap()
    bqk_d = nc.dram_tensor("b_qk", [P, 4], F32, kind="ExternalInput").ap()
    bv_d = nc.dram_tensor("b_v", [1, CLA], F16, kind="ExternalInput").ap()
    bprT_d = nc.dram_tensor("b_prT", [P, C // P], F32, kind="ExternalInput").ap()
    qkw_d = nc.dram_tensor("qkw", [P, 4], F32, kind="ExternalInput").ap()
    outT_d = nc.dram_tensor("outT", [C, n], F32, kind="ExternalOutput").ap()

    with tile.TileContext(nc) as tc, ExitStack() as ctx:
        con = ctx.enter_context(tc.tile_pool(name="con", bufs=1))
        wp = ctx.enter_context(tc.tile_pool(name="wp", bufs=1))
        qk = ctx.enter_context(tc.tile_pool(name="qk", bufs=1))
        vp = ctx.enter_context(tc.tile_pool(name="vp", bufs=1))
        xp = ctx.enter_context(tc.tile_pool(name="xp", bufs=1))
        sqp = ctx.enter_context(tc.tile_pool(name="sqp", bufs=4))
        rp = ctx.enter_context(tc.tile_pool(name="rp", bufs=2))
        ptp = ctx.enter_context(tc.tile_pool(name="ptp", bufs=1))
        atp = ctx.enter_context(tc.tile_pool(name="atp", bufs=1))
        rp2 = ctx.enter_context(tc.tile_pool(name="rp2", bufs=2))
        osp = ctx.enter_context(tc.tile_pool(name="osp", bufs=4))
        ps = ctx.enter_context(tc.tile_pool(name="ps", bufs=1, space="PSUM"))

        bv_sb = con.tile([1, CLA], F16, tag="bv")
        bqk_sb = con.tile([P, 4], F32, tag="bqk")
        qkw_sb = con.tile([P, 4], F32, tag="qkw")
        bprT_sb = con.tile([P, C // P], F32, tag="bprT")

        # ---- constants (compute engines; no DMA) ----
        ones_r = con.tile([1, P], F16, tag="onesr")      # lhsT for bias bcasts
        nc.vector.memset(ones_r[:], 1.0)
        ones_m = con.tile([P, P], F16, tag="onesm")      # lhsT for sumsq
        nc.vector.memset(ones_m[:], 1.0)
        eps_sb = con.tile([P, 1], F32, tag="eps")
        nc.vector.memset(eps_sb[:], EPS)
        # ---- weight tiles (DMAs interleaved with x below) ----
        wv_sb = [wp.tile([P, CLA], F16, tag=f"wv{k}", name=f"wv{k}") for k in range(kt)]
        wqk_sb = [wp.tile([P, 2 * CL], F16, tag=f"wqk{k}", name=f"wqk{k}") for k in range(kt)]
        wpr_sb = [wp.tile([P, C], F16, tag=f"wpr{t}", name=f"wpr{t}") for t in range(CL // P)]

        # rotating x tiles: 2 j-blocks in flight per k
        def x_tile(j, k):
            return xp.tile([P, nq], F16, tag=f"xt{k}", bufs=2, name=f"xt{k}_{j}")

        # persistent attention operands
        qkT = [qk.tile([P, n], F16, tag=f"qkT{m}", name=f"qkT{m}") for m in range(4)]
        v_aug = [vp.tile([P, CLA], F16, tag=f"va{i}", name=f"va{i}") for i in range(nb)]
        attnT = [atp.tile([P, n], F16, tag=f"at{t}", name=f"at{t}") for t in range(HL // HPB)]

        # startup: v weights + x(j=0) interleaved, consts slotted in after
        # the first pair, then qk/proj weights.
        xs = {}
        for k in range(kt):
            nc.sync.dma_start(wv_sb[k][:], wv_d[k * P:(k + 1) * P, :])
            t = x_tile(0, k)
            nc.sync.dma_start(t[:], xT_d[k * P:(k + 1) * P, 0:nq])
            xs[(0, k)] = t
            if k == 0:
                nc.sync.dma_start(bv_sb[:], bv_d[:])
        nc.sync.dma_start(bqk_sb[:], bqk_d[:])
        nc.sync.dma_start(qkw_sb[:], qkw_d[:])
        for k in range(kt):
            nc.sync.dma_start(wqk_sb[k][:], wqk_d[k * P:(k + 1) * P, :])
        nc.sync.dma_start(bprT_sb[:], bprT_d[:])
        for t in range(CL // P):
            nc.sync.dma_start(wpr_sb[t][:], wpr_d[t * P:(t + 1) * P, :])

        # ---- stage 1+2, pipelined per token block j ----
        for j in range(nj):
            js = slice(j * nq, (j + 1) * nq)
            if j + 1 < nj:
                for k in range(kt):
                    t = x_tile(j + 1, k)
                    nc.sync.dma_start(
                        t[:], xT_d[k * P:(k + 1) * P,
                                   (j + 1) * nq:(j + 2) * nq])
                    xs[(j + 1, k)] = t

            # v for this block's nq//P key blocks, directly in augmented
            # layout (wv_aug has zero columns at the ones positions; the
            # bias row carries 1.0 there)
            for i in range(j * (nq // P), (j + 1) * (nq // P)):
                ioff = i * P - j * nq
                acc = ps.tile([P, CLA], F32, tag="s2", bufs=2, name="vacc")
                nc.tensor.matmul(acc[:], ones_r[0:1, 0:P], bv_sb[:],
                                 start=True, stop=False)
                for k in range(kt):
                    nc.tensor.matmul(
                        acc[:], xs[(j, k)][:, ioff:ioff + P], wv_sb[k][:],
                        start=False, stop=(k == kt - 1))
                nc.vector.tensor_copy(v_aug[i][:], acc[:])

            # qk projection + rmsnorm, k-heads (m=2,3) first: attention's
            # kT dependency clears while the q half of the last block
            # computes
            def emit_qk(m):
                acc = ps.tile([P, nq], F32, tag="s2", bufs=2, name="qacc")
                for k in range(kt):
                    nc.tensor.matmul(
                        acc[:], wqk_sb[k][:, m * P:(m + 1) * P],
                        xs[(j, k)][:, :], start=(k == 0), stop=(k == kt - 1))
                nc.vector.tensor_scalar_add(
                    qkT[m][:, js], acc[:], bqk_sb[:, m:m + 1])

            def emit_rms_for(jr):
                jrs = slice(jr * nq, (jr + 1) * nq)

                def emit_rms(m):
                    # both heads of the m-slice share one ssq tile: the
                    # h2=1 sumsq matmul is tile-positioned at partition 64
                    sq = sqp.tile([P, nq], F16, tag="sq", name="sq")
                    nc.vector.tensor_mul(sq[:], qkT[m][:, jrs], qkT[m][:, jrs])
                    ssq = ps.tile([P, nq], F32, tag="oas", bufs=4, name="ssq")
                    for h2 in range(HPB):
                        pr = slice(h2 * D, (h2 + 1) * D)
                        nc.tensor.matmul(ssq[pr, :], ones_m[pr, 0:D],
                                         sq[pr, :], start=True, stop=True)
                    rms = rp.tile([P, nq], F32, tag="rms", bufs=4, name="rms")
                    nc.scalar.activation(rms[:], ssq[:], AF.Sqrt,
                                         scale=1.0 / D, bias=eps_sb[:, 0:1])
                    rec = rp.tile([P, nq], F32, tag="rec", bufs=4, name="rec")
                    nc.vector.reciprocal_approx_fast(rec[:], rms[:])
                    nc.vector.scalar_tensor_tensor(
                        qkT[m][:, jrs], qkT[m][:, jrs], qkw_sb[:, m:m + 1],
                        rec[:], op0=mybir.AluOpType.mult,
                        op1=mybir.AluOpType.mult)

                return emit_rms

            for m in (2, 3, 0, 1):
                emit_qk(m)
            if j > 0:
                for m in (2, 3, 0, 1):
                    emit_rms_for(j - 1)(m)

        for m in (2, 3, 0, 1):
            emit_rms_for(nj - 1)(m)

        # ---- attention: S(cur) || AV(prev), then norm(prev2) + proj ----
        units = [(j, hp) for j in range(nj) for hp in range(HL // HPB)]
        BLK = min(4, nb)

        def emit_s(u, i):
            j, hp = u
            js = slice(j * nq, (j + 1) * nq)
            qm, km = hp, 2 + hp
            s2 = ps.tile([P, 2 * nq], F32, tag="s2", bufs=2, name="s2")
            for sub in range(HPB):
                pr = slice(sub * D, (sub + 1) * D)
                nc.tensor.matmul(
                    s2[:, sub * nq:(sub + 1) * nq],
                    qkT[km][pr, i * P:(i + 1) * P], qkT[qm][pr, js],
                    start=True, stop=True)
            pt = ptp.tile([P, 2 * nq], F16, tag="pt", bufs=20, name="pt")
            nc.scalar.activation(pt[:], s2[:], AF.Exp, scale=SCALE)
            return pt

        def emit_av(u, oas, pts, i):
            j, hp = u
            for sub in range(HPB):
                h = hp * HPB + sub
                nc.tensor.matmul(
                    oas[sub][0:VW, :], v_aug[i][:, h * VW:(h + 1) * VW],
                    pts[i][:, sub * nq:(sub + 1) * nq],
                    start=(i == 0), stop=(i == nb - 1))

        def emit_norm(u, oas):
            # Softmax denominators ride in row 64 of each AV output. Copy
            # them to 1-partition rows, PE-broadcast each to 64 partitions
            # (both broadcasts share one PSUM slot; base-0 outputs only —
            # nonzero output tile positions with K=1 fail the ISA check),
            # reciprocal into SBUF, then normalize into attnT.
            j, hp = u
            js = slice(j * nq, (j + 1) * nq)
            for sub in range(HPB):
                sums = rp2.tile([1, nq], F16, tag=f"sums{sub}", name="sums")
                nc.vector.tensor_copy(sums[:], oas[sub][D:VW, :])
                bcs = ps.tile([D, nq], F32, tag="s2", bufs=2, name="bcs")
                nc.tensor.matmul(bcs[:], ones_r[0:1, 0:D], sums[:],
                                 start=True, stop=True)
                recn = rp2.tile([D, nq], F32, tag=f"recn{sub}", name="recn")
                nc.vector.reciprocal_approx_fast(recn[:], bcs[:])
                pr = slice(sub * D, (sub + 1) * D)
                nc.vector.tensor_mul(attnT[hp][pr, js], oas[sub][0:D, :],
                                     recn[:])

        def emit_proj(j):
            js = slice(j * nq, (j + 1) * nq)
            for m in range(C // P):
                acc = ps.tile([P, nq], F32, tag="s2", bufs=2, name="pacc")
                for t in range(CL // P):
                    nc.tensor.matmul(
                        acc[:], wpr_sb[t][:, m * P:(m + 1) * P],
                        attnT[t][:, js], start=(t == 0), stop=(t == CL // P - 1))
                ost = osp.tile([P, nq], F32, tag="ost", name="ost")
                nc.vector.tensor_scalar_add(ost[:], acc[:], bprT_sb[:, m:m + 1])
                nc.sync.dma_start(outT_d[m * P:(m + 1) * P, js], ost[:])

        prev = None    # (unit, oas, pts)
        prev2 = None
        for idx in range(len(units) + 2):
            cur = units[idx] if idx < len(units) else None
            # norm/proj of prev2 first: their reads gate the slot reuse of
            # this iteration's AV writes, so they must precede them in
            # scheduler priority (emitting them later deadlocks the
            # in-order PE queue)
            if prev2 is not None:
                emit_norm(prev2[0], prev2[1])
                j2, hp2 = prev2[0]
                if hp2 == HL // HPB - 1:
                    emit_proj(j2)
            oas_prev = None
            if prev is not None:
                oas_prev = [ps.tile([P, nq], F32, tag="oas", bufs=4,
                                    name=f"oa{s_}") for s_ in range(HPB)]
            pts = {}
            for ib in range((nb + BLK - 1) // BLK):
                blk = range(ib * BLK, min((ib + 1) * BLK, nb))
                if cur is not None:
                    for i in blk:
                        pts[i] = emit_s(cur, i)
                if prev is not None:
                    for i in blk:
                        emit_av(prev[0], oas_prev, prev[2], i)
            prev2 = (prev[0], oas_prev) if prev is not None else None
            prev = (cur, None, pts) if cur is not None else None

    nc.compile()
    return nc


_NC_CACHE = {}


def _get_nc(n=N, nq=NQ):
    key = (n, nq)
    if key not in _NC_CACHE:
        _NC_CACHE[key] = build(n, nq)
    return _NC_CACHE[key]


def make_in_maps(x, w_qkv, b_qkv, q_w, k_w, w_proj, b_proj):
    """Shard full inputs into per-core in_maps (host side)."""
    in_maps = []
    for cid in range(NCORES):
        b, g = cid // GROUPS, cid % GROUPS
        c0 = g * CL
        xT = np.ascontiguousarray(x[b].T)
        w_qk = np.ascontiguousarray(
            np.concatenate([w_qkv[:, c0:c0 + CL],
                            w_qkv[:, C + c0:C + c0 + CL]], axis=1))
        # v weights/bias in augmented layout: zero weight columns and a
        # 1.0 bias at each per-head ones position
        wv_aug = np.zeros((C, CLA), dtype=np.float32)
        bv_aug = np.zeros((1, CLA), dtype=np.float32)
        for h in range(HL):
            src = 2 * C + c0 + h * D
            wv_aug[:, h * VW:h * VW + D] = w_qkv[:, src:src + D]
            bv_aug[0, h * VW:h * VW + D] = b_qkv[src:src + D]
            bv_aug[0, h * VW + D] = 1.0
        w_pr = np.ascontiguousarray(w_proj[c0:c0 + CL, :])
        b_qk = np.stack([b_qkv[c0 + m * P:c0 + (m + 1) * P] for m in range(2)]
                        + [b_qkv[C + c0 + m * P:C + c0 + (m + 1) * P]
                           for m in range(2)], axis=1)
        # host gather sums GROUPS partials per batch; split the bias so it
        # lands exactly once
        b_prT = np.ascontiguousarray((b_proj / GROUPS).reshape(C // P, P).T)
        qkw = np.stack([np.tile(q_w, HPB), np.tile(q_w, HPB),
                        np.tile(k_w, HPB), np.tile(k_w, HPB)], axis=1)
        in_maps.append({
            "xT": xT.astype(np.float16),
            "w_qk": w_qk.astype(np.float16),
            "w_v": wv_aug.astype(np.float16),
            "w_pr": w_pr.astype(np.float16),
            "b_qk": np.ascontiguousarray(b_qk).astype(np.float32),
            "b_v": bv_aug.astype(np.float16),
            "b_prT": b_prT.astype(np.float32),
            "qkw": np.ascontiguousarray(qkw).astype(np.float32),
        })
    return in_maps


def kernel(x, w_qkv, b_qkv, q_w, k_w, w_proj, b_proj, _trace=False):
    x = np.asarray(x)
    n = x.shape[1]
    nc = _get_nc(n, NQ if n % NQ == 0 else P)
    in_maps = make_in_maps(np.asarray(x, np.float32), np.asarray(w_qkv, np.float32),
                           np.asarray(b_qkv, np.float32), np.asarray(q_w, np.float32),
                           np.asarray(k_w, np.float32), np.asarray(w_proj, np.float32),
                           np.asarray(b_proj, np.float32))
    res = run_bass_kernel_spmd(nc, in_maps, core_ids=list(range(NCORES)),
                               trace=_trace)
    # TP unshard: sum the 4 head-group partials per batch, transpose, stack
    out = np.stack([
        sum(res.results[b * GROUPS + g]["outT"] for g in range(GROUPS)).T
        for b in range(B)
    ]).astype(np.float32)
    if _trace:
        return out, res
    return out


# revision 12
# speedup vs baseline: 1.1838x; 1.0748x over previous
"""Multi-head attention (B=2, N=2048, C=1024, H=16, qk-RMSNorm) on 8 TRN2 cores.

v3 of the kernel. Same sharding as baseline (TP over 4 head groups x DP
over batch; host sums the 4 w_proj partials per batch) and the same
attention pipeline (S(cur) || AV(prev), then norm(prev2) + proj).

Changes vs v2 (299.98us):
- fp16 everywhere 16-bit data goes: x, w_qkv, w_proj, qkT, pt, v_aug,
  attnT, sums. Same PE rate as fp32r/bf16, half the DMA + LDWEIGHTS
  bytes of the fp32r GEMM weights, and 4x less quantization noise than
  bf16 (fp16 has 10 mantissa bits vs bf16's 8; all magnitudes here are
  well inside fp16 range: |logits*scale| < ~6 -> pt < ~450 < 65504).
- v_aug is produced directly by the v GEMM: w_v is host-padded to
  [C, HL*65] with zero columns at the per-head ones positions and the
  bias row carries 1.0 there, so acc = x@wv_aug + ones*bv_aug lands in
  the augmented layout. One DVE copy per key block replaces the 4
  scalar.copy + 4 gpsimd ones-writes of v2 (~30us of ACT/Pool work).
- RMSNorm: the two heads of an m-slice share one ssq PSUM tile (the
  second head's sumsq matmul is tile-positioned at partition 64), so
  sqrt / reciprocal / scale-mul run once per m-slice on [128, nq]
  instead of per head on [64, nq]: halves those ACT/DVE instr counts.
  sq is computed from the fp16 qkT in SBUF (2x DVE mode) instead of
  from PSUM.

PSUM budget unchanged: tag "s2" [128,1024] bufs=2, tag "oas" [128,512]
bufs=4.
"""

import sys

if "/opt/trn_rl_repo" not in sys.path:
    sys.path.insert(0, "/opt/trn_rl_repo")

from contextlib import ExitStack

import numpy as np

import concourse.mybir as mybir
import concourse.tile as tile
from concourse import bacc
from concourse.bass_utils import run_bass_kernel_spmd

F32 = mybir.dt.float32
F16 = mybir.dt.float16
AF = mybir.ActivationFunctionType

B, N, C, H = 2, 2048, 1024, 16
D = C // H          # 64
EPS = 1e-6
NCORES = 8
GROUPS = 4          # head groups (cores per batch)
HL = H // GROUPS    # heads per core = 4
CL = HL * D         # local channels = 256
SCALE = D ** -0.5   # 0.125

P = 128             # partition dim
KT = C // P         # 8 contraction tiles over C
NQ = 512            # query/token block
HPB = P // D        # heads per 128-channel block = 2
VW = D + 1          # 65: v columns + ones column
CLA = HL * VW       # 260: augmented v width


def build(n=N, nq=NQ):
    nb = n // P          # key blocks of 128
    nj = n // nq         # token blocks of nq
    kt = KT

    nc = bacc.Bacc("TRN2", target_bir_lowering=False, debug=False,
                   num_devices=NCORES)

    xT_d = nc.dram_tensor("xT", [C, n], F16, kind="ExternalInput").ap()
    wqk_d = nc.dram_tensor("w_qk", [C, 2 * CL], F16, kind="ExternalInput").ap()
    wv_d = nc.dram_tensor("w_v", [C, CL], F16, kind="ExternalInput").ap()
    wpr_d = nc.dram_tensor("w_pr", [CL, C], F16, kind="ExternalInput").ap()
    bqk_d = nc.dram_tensor("b_qk", [P, 4], F32, kind="ExternalInput").ap()
    qkw_d = nc.dram_tensor("qkw", [P, 4], F32, kind="ExternalInput").ap()
    outT_d = nc.dram_tensor("outT", [C, n], F16, kind="ExternalOutput").ap()

    with tile.TileContext(nc) as tc, ExitStack() as ctx:
        con = ctx.enter_context(tc.tile_pool(name="con", bufs=1))
        wp = ctx.enter_context(tc.tile_pool(name="wp", bufs=1))
        qk = ctx.enter_context(tc.tile_pool(name="qk", bufs=1))
        vp = ctx.enter_context(tc.tile_pool(name="vp", bufs=1))
        xp = ctx.enter_context(tc.tile_pool(name="xp", bufs=1))
        sqp = ctx.enter_context(tc.tile_pool(name="sqp", bufs=4))
        rp = ctx.enter_context(tc.tile_pool(name="rp", bufs=2))
        ptp = ctx.enter_context(tc.tile_pool(name="ptp", bufs=1))
        atp = ctx.enter_context(tc.tile_pool(name="atp", bufs=1))
        rp2 = ctx.enter_context(tc.tile_pool(name="rp2", bufs=2))
        osp = ctx.enter_context(tc.tile_pool(name="osp", bufs=4))
        ps = ctx.enter_context(tc.tile_pool(name="ps", bufs=1, space="PSUM"))

        bqk_sb = con.tile([P, 4], F32, tag="bqk")
        qkw_sb = con.tile([P, 4], F32, tag="qkw")

        # ---- constants (compute engines; no DMA) ----
        ones_r = con.tile([1, P], F16, tag="onesr")      # lhsT for bias bcasts
        nc.vector.memset(ones_r[:], 1.0)
        ones_m = con.tile([P, P], F16, tag="onesm")      # lhsT for sumsq
        nc.vector.memset(ones_m[:], 1.0)
        eps_sb = con.tile([P, 1], F32, tag="eps")
        nc.vector.memset(eps_sb[:], EPS)
        # ---- weight tiles (DMAs interleaved with x below) ----
        wv_sb = [wp.tile([P, CL], F16, tag=f"wv{k}", name=f"wv{k}") for k in range(kt)]
        wqk_sb = [wp.tile([P, 2 * CL], F16, tag=f"wqk{k}", name=f"wqk{k}") for k in range(kt)]
        wpr_sb = [wp.tile([P, C], F16, tag=f"wpr{t}", name=f"wpr{t}") for t in range(CL // P)]

        # rotating x tiles: 2 j-blocks in flight per k
        def x_tile(j, k):
            return xp.tile([P, nq], F16, tag=f"xt{k}", bufs=2, name=f"xt{k}_{j}")

        # persistent attention operands
        qkT = [qk.tile([P, n], F16, tag=f"qkT{m}", name=f"qkT{m}") for m in range(4)]
        v_aug = [vp.tile([P, HL, VW], F16, tag=f"va{i}", name=f"va{i}") for i in range(nb)]
        attnT = [atp.tile([P, n], F16, tag=f"at{t}", name=f"at{t}") for t in range(HL // HPB)]

        # ones columns of v_aug are written once here; the per-block drains
        # only touch the v columns (strided copy)
        ones_c = con.tile([P, HL, 1], F16, tag="onesc")
        nc.gpsimd.memset(ones_c[:], 1.0)
        for i in range(nb):
            nc.gpsimd.tensor_copy(v_aug[i][:, :, D:VW], ones_c[:])

        # startup: v weights + x(j=0) interleaved, consts slotted in after
        # the first pair, then qk/proj weights.
        xs = {}
        for k in range(kt):
            nc.sync.dma_start(wv_sb[k][:], wv_d[k * P:(k + 1) * P, :])
            t = x_tile(0, k)
            nc.sync.dma_start(t[:], xT_d[k * P:(k + 1) * P, 0:nq])
            xs[(0, k)] = t
        nc.sync.dma_start(bqk_sb[:], bqk_d[:])
        nc.sync.dma_start(qkw_sb[:], qkw_d[:])
        for k in range(kt):
            nc.sync.dma_start(wqk_sb[k][:], wqk_d[k * P:(k + 1) * P, :])
        for t in range(CL // P):
            nc.sync.dma_start(wpr_sb[t][:], wpr_d[t * P:(t + 1) * P, :])

        # ---- stage 1+2, pipelined per token block j ----
        for j in range(nj):
            js = slice(j * nq, (j + 1) * nq)
            if j + 1 < nj:
                for k in range(kt):
                    t = x_tile(j + 1, k)
                    nc.sync.dma_start(
                        t[:], xT_d[k * P:(k + 1) * P,
                                   (j + 1) * nq:(j + 2) * nq])
                    xs[(j + 1, k)] = t

            # v for this block's nq//P key blocks (b_v is folded into the
            # host-side output bias since attention rows sum to 1); the
            # strided drain leaves the ones columns intact
            for i in range(j * (nq // P), (j + 1) * (nq // P)):
                ioff = i * P - j * nq
                acc = ps.tile([P, HL, D], F32, tag="s2", bufs=2, name="vacc")
                for k in range(kt):
                    nc.tensor.matmul(
                        acc[:], xs[(j, k)][:, ioff:ioff + P], wv_sb[k][:],
                        start=(k == 0), stop=(k == kt - 1))
                nc.vector.tensor_copy(v_aug[i][:, :, 0:D], acc[:])

            # qk projection + rmsnorm, k-heads (m=2,3) first: attention's
            # kT dependency clears while the q half of the last block
            # computes
            def emit_qk(m):
                acc = ps.tile([P, nq], F32, tag="s2", bufs=2, name="qacc")
                for k in range(kt):
                    nc.tensor.matmul(
                        acc[:], wqk_sb[k][:, m * P:(m + 1) * P],
                        xs[(j, k)][:, :], start=(k == 0), stop=(k == kt - 1))
                nc.vector.tensor_scalar_add(
                    qkT[m][:, js], acc[:], bqk_sb[:, m:m + 1])

            def emit_rms_for(jr):
                jrs = slice(jr * nq, (jr + 1) * nq)

                def emit_rms(m):
                    # both heads of the m-slice share one ssq tile: the
                    # h2=1 sumsq matmul is tile-positioned at partition 64
                    sq = sqp.tile([P, nq], F16, tag="sq", name="sq")
                    nc.vector.tensor_mul(sq[:], qkT[m][:, jrs], qkT[m][:, jrs])
                    ssq = ps.tile([P, nq], F32, tag="oas", bufs=4, name="ssq")
                    for h2 in range(HPB):
                        pr = slice(h2 * D, (h2 + 1) * D)
                        nc.tensor.matmul(ssq[pr, :], ones_m[pr, 0:D],
                                         sq[pr, :], start=True, stop=True)
                    rms = rp.tile([P, nq], F32, tag="rms", bufs=4, name="rms")
                    nc.scalar.activation(rms[:], ssq[:], AF.Sqrt,
                                         scale=1.0 / D, bias=eps_sb[:, 0:1])
                    rec = rp.tile([P, nq], F32, tag="rec", bufs=4, name="rec")
                    nc.vector.reciprocal_approx_fast(rec[:], rms[:])
                    nc.vector.scalar_tensor_tensor(
                        qkT[m][:, jrs], qkT[m][:, jrs], qkw_sb[:, m:m + 1],
                        rec[:], op0=mybir.AluOpType.mult,
                        op1=mybir.AluOpType.mult)

                return emit_rms

            if j < nj - 1:
                for m in (2, 3, 0, 1):
                    emit_qk(m)
                if j > 0:
                    for m in (2, 3, 0, 1):
                        emit_rms_for(j - 1)(m)
            else:
                # last block: norm the k heads before the q projections so
                # the attention's kT dependency clears under the q matmuls
                emit_qk(2)
                emit_qk(3)
                if j > 0:
                    for m in (2, 3, 0, 1):
                        emit_rms_for(j - 1)(m)
                emit_rms_for(j)(2)
                emit_rms_for(j)(3)
                emit_qk(0)
                emit_qk(1)

        emit_rms_for(nj - 1)(0)
        emit_rms_for(nj - 1)(1)

        # ---- attention: S(cur) || AV(prev), then norm(prev2) + proj ----
        units = [(j, hp) for j in range(nj) for hp in range(HL // HPB)]
        BLK = min(4, nb)

        def emit_s(u, i):
            j, hp = u
            js = slice(j * nq, (j + 1) * nq)
            qm, km = hp, 2 + hp
            s2 = ps.tile([P, 2 * nq], F32, tag="s2", bufs=2, name="s2")
            for sub in range(HPB):
                pr = slice(sub * D, (sub + 1) * D)
                nc.tensor.matmul(
                    s2[:, sub * nq:(sub + 1) * nq],
                    qkT[km][pr, i * P:(i + 1) * P], qkT[qm][pr, js],
                    start=True, stop=True)
            pt = ptp.tile([P, 2 * nq], F16, tag="pt", bufs=20, name="pt")
            nc.scalar.activation(pt[:], s2[:], AF.Exp, scale=SCALE)
            return pt

        def emit_av(u, oas, pts, i):
            j, hp = u
            for sub in range(HPB):
                h = hp * HPB + sub
                nc.tensor.matmul(
                    oas[sub][0:VW, :], v_aug[i][:, h:h + 1, :],
                    pts[i][:, sub * nq:(sub + 1) * nq],
                    start=(i == 0), stop=(i == nb - 1))

        def emit_norm(u, oas):
            # Softmax denominators ride in row 64 of each AV output. Copy
            # them to 1-partition rows, PE-broadcast each to 64 partitions
            # (both broadcasts share one PSUM slot; base-0 outputs only —
            # nonzero output tile positions with K=1 fail the ISA check),
            # reciprocal into SBUF, then normalize into attnT.
            j, hp = u
            js = slice(j * nq, (j + 1) * nq)
            for sub in range(HPB):
                sums = rp2.tile([1, nq], F16, tag=f"sums{sub}", name="sums")
                nc.vector.tensor_copy(sums[:], oas[sub][D:VW, :])
                bcs = ps.tile([D, nq], F32, tag="s2", bufs=2, name="bcs")
                nc.tensor.matmul(bcs[:], ones_r[0:1, 0:D], sums[:],
                                 start=True, stop=True)
                recn = rp2.tile([D, nq], F32, tag=f"recn{sub}", name="recn")
                nc.vector.reciprocal_approx_fast(recn[:], bcs[:])
                pr = slice(sub * D, (sub + 1) * D)
                nc.vector.tensor_mul(attnT[hp][pr, js], oas[sub][0:D, :],
                                     recn[:])

        def emit_proj_m(j, m):
            # b_proj (and the folded b_v term) are added on the host, so
            # the drain is a plain PSUM->fp16 copy
            js = slice(j * nq, (j + 1) * nq)
            acc = ps.tile([P, nq], F32, tag="s2", bufs=2, name="pacc")
            for t in range(CL // P):
                nc.tensor.matmul(
                    acc[:], wpr_sb[t][:, m * P:(m + 1) * P],
                    attnT[t][:, js], start=(t == 0), stop=(t == CL // P - 1))
            ost = osp.tile([P, nq], F16, tag="ost", name="ost")
            nc.vector.tensor_copy(ost[:], acc[:])
            nc.sync.dma_start(outT_d[m * P:(m + 1) * P, js], ost[:])

        nchunks = (nb + BLK - 1) // BLK
        mpc = (C // P) // nchunks      # proj m-steps per chunk
        prev = None    # (unit, oas, pts)
        prev2 = None
        for idx in range(len(units) + 2):
            cur = units[idx] if idx < len(units) else None
            # norm of prev2 first: its reads gate the slot reuse of this
            # iteration's AV writes, so it must precede them in scheduler
            # priority (emitting it later deadlocks the in-order PE queue)
            proj_j = None
            if prev2 is not None:
                emit_norm(prev2[0], prev2[1])
                j2, hp2 = prev2[0]
                if hp2 == HL // HPB - 1:
                    proj_j = j2
            oas_prev = None
            if prev is not None:
                oas_prev = [ps.tile([P, nq], F32, tag="oas", bufs=4,
                                    name=f"oa{s_}") for s_ in range(HPB)]
            pts = {}
            for ib in range(nchunks):
                blk = range(ib * BLK, min((ib + 1) * BLK, nb))
                if cur is not None:
                    for i in blk:
                        pts[i] = emit_s(cur, i)
                if prev is not None:
                    for i in blk:
                        emit_av(prev[0], oas_prev, prev[2], i)
                # proj of the finished block rides along, a couple of
                # m-slices per chunk, so its PSUM slot rotation (and the
                # drain DVE work) spreads across the unit instead of
                # stalling the PE in one burst
                if proj_j is not None:
                    for m in range(ib * mpc, (ib + 1) * mpc):
                        emit_proj_m(proj_j, m)
            if proj_j is not None:
                for m in range(nchunks * mpc, C // P):
                    emit_proj_m(proj_j, m)
            prev2 = (prev[0], oas_prev) if prev is not None else None
            prev = (cur, None, pts) if cur is not None else None

    nc.compile()
    return nc


_NC_CACHE = {}


def _get_nc(n=N, nq=NQ):
    key = (n, nq)
    if key not in _NC_CACHE:
        _NC_CACHE[key] = build(n, nq)
    return _NC_CACHE[key]


def make_in_maps(x, w_qkv, b_qkv, q_w, k_w, w_proj, b_proj):
    """Shard full inputs into per-core in_maps (host side)."""
    in_maps = []
    for cid in range(NCORES):
        b, g = cid // GROUPS, cid % GROUPS
        c0 = g * CL
        xT = np.ascontiguousarray(x[b].T)
        w_qk = np.ascontiguousarray(
            np.concatenate([w_qkv[:, c0:c0 + CL],
                            w_qkv[:, C + c0:C + c0 + CL]], axis=1))
        w_v = np.ascontiguousarray(w_qkv[:, 2 * C + c0:2 * C + c0 + CL])
        w_pr = np.ascontiguousarray(w_proj[c0:c0 + CL, :])
        b_qk = np.stack([b_qkv[c0 + m * P:c0 + (m + 1) * P] for m in range(2)]
                        + [b_qkv[C + c0 + m * P:C + c0 + (m + 1) * P]
                           for m in range(2)], axis=1)
        qkw = np.stack([np.tile(q_w, HPB), np.tile(q_w, HPB),
                        np.tile(k_w, HPB), np.tile(k_w, HPB)], axis=1)
        in_maps.append({
            "xT": xT.astype(np.float16),
            "w_qk": w_qk.astype(np.float16),
            "w_v": w_v.astype(np.float16),
            "w_pr": w_pr.astype(np.float16),
            "b_qk": np.ascontiguousarray(b_qk).astype(np.float32),
            "qkw": np.ascontiguousarray(qkw).astype(np.float32),
        })
    return in_maps


def kernel(x, w_qkv, b_qkv, q_w, k_w, w_proj, b_proj, _trace=False):
    x = np.asarray(x)
    n = x.shape[1]
    nc = _get_nc(n, NQ if n % NQ == 0 else P)
    in_maps = make_in_maps(np.asarray(x, np.float32), np.asarray(w_qkv, np.float32),
                           np.asarray(b_qkv, np.float32), np.asarray(q_w, np.float32),
                           np.asarray(k_w, np.float32), np.asarray(w_proj, np.float32),
                           np.asarray(b_proj, np.float32))
    res = run_bass_kernel_spmd(nc, in_maps, core_ids=list(range(NCORES)),
                               trace=_trace)
    # TP unshard: sum the 4 head-group fp16 partials per batch, transpose,
    # and add the host-folded bias (b_proj plus b_v @ w_proj: attention
    # rows sum to 1, so the v bias is additive on attn_out)
    bias = (np.asarray(b_proj, np.float32)
            + np.asarray(b_qkv[2 * C:3 * C], np.float32)
            @ np.asarray(w_proj, np.float32))
    out = np.stack([
        sum(res.results[b * GROUPS + g]["outT"].astype(np.float32)
            for g in range(GROUPS)).T + bias
        for b in range(B)
    ]).astype(np.float32)
    if _trace:
        return out, res
    return out


# revision 16
# speedup vs baseline: 1.2250x; 1.0348x over previous
"""Multi-head attention (B=2, N=2048, C=1024, H=16, qk-RMSNorm) on 8 TRN2 cores.

v3 of the kernel. Same sharding as baseline (TP over 4 head groups x DP
over batch; host sums the 4 w_proj partials per batch) and the same
attention pipeline (S(cur) || AV(prev), then norm(prev2) + proj).

Changes vs v2 (299.98us):
- fp16 everywhere 16-bit data goes: x, w_qkv, w_proj, qkT, pt, v_aug,
  attnT, sums. Same PE rate as fp32r/bf16, half the DMA + LDWEIGHTS
  bytes of the fp32r GEMM weights, and 4x less quantization noise than
  bf16 (fp16 has 10 mantissa bits vs bf16's 8; all magnitudes here are
  well inside fp16 range: |logits*scale| < ~6 -> pt < ~450 < 65504).
- v_aug is produced directly by the v GEMM: w_v is host-padded to
  [C, HL*65] with zero columns at the per-head ones positions and the
  bias row carries 1.0 there, so acc = x@wv_aug + ones*bv_aug lands in
  the augmented layout. One DVE copy per key block replaces the 4
  scalar.copy + 4 gpsimd ones-writes of v2 (~30us of ACT/Pool work).
- RMSNorm: the two heads of an m-slice share one ssq PSUM tile (the
  second head's sumsq matmul is tile-positioned at partition 64), so
  sqrt / reciprocal / scale-mul run once per m-slice on [128, nq]
  instead of per head on [64, nq]: halves those ACT/DVE instr counts.
  sq is computed from the fp16 qkT in SBUF (2x DVE mode) instead of
  from PSUM.

PSUM budget unchanged: tag "s2" [128,1024] bufs=2, tag "oas" [128,512]
bufs=4.
"""

import sys

if "/opt/trn_rl_repo" not in sys.path:
    sys.path.insert(0, "/opt/trn_rl_repo")

from contextlib import ExitStack

import numpy as np

import concourse.mybir as mybir
import concourse.tile as tile
from concourse import bacc
from concourse.bass_utils import run_bass_kernel_spmd

F32 = mybir.dt.float32
F16 = mybir.dt.float16
AF = mybir.ActivationFunctionType

B, N, C, H = 2, 2048, 1024, 16
D = C // H          # 64
EPS = 1e-6
NCORES = 8
GROUPS = 4          # head groups (cores per batch)
HL = H // GROUPS    # heads per core = 4
CL = HL * D         # local channels = 256
SCALE = D ** -0.5   # 0.125

P = 128             # partition dim
KT = C // P         # 8 contraction tiles over C
NQ = 512            # query/token block
HPB = P // D        # heads per 128-channel block = 2
VW = D + 1          # 65: v columns + ones column
CLA = HL * VW       # 260: augmented v width


def build(n=N, nq=NQ):
    nb = n // P          # key blocks of 128
    nj = n // nq         # token blocks of nq
    kt = KT

    nc = bacc.Bacc("TRN2", target_bir_lowering=False, debug=False,
                   num_devices=NCORES)

    xT_d = nc.dram_tensor("xT", [C, n], F16, kind="ExternalInput").ap()
    wqk_d = nc.dram_tensor("w_qk", [C, 2 * CL], F16, kind="ExternalInput").ap()
    wv_d = nc.dram_tensor("w_v", [C, CL], F16, kind="ExternalInput").ap()
    wpr_d = nc.dram_tensor("w_pr", [CL, C], F16, kind="ExternalInput").ap()
    bqk_d = nc.dram_tensor("b_qk", [P, 4], F32, kind="ExternalInput").ap()
    qkw_d = nc.dram_tensor("qkw", [P, 4], F32, kind="ExternalInput").ap()
    outT_d = nc.dram_tensor("outT", [C, n], F16, kind="ExternalOutput").ap()

    with tile.TileContext(nc) as tc, ExitStack() as ctx:
        con = ctx.enter_context(tc.tile_pool(name="con", bufs=1))
        wp = ctx.enter_context(tc.tile_pool(name="wp", bufs=1))
        qk = ctx.enter_context(tc.tile_pool(name="qk", bufs=1))
        vp = ctx.enter_context(tc.tile_pool(name="vp", bufs=1))
        xp = ctx.enter_context(tc.tile_pool(name="xp", bufs=1))
        sqp = ctx.enter_context(tc.tile_pool(name="sqp", bufs=4))
        rp = ctx.enter_context(tc.tile_pool(name="rp", bufs=2))
        ptp = ctx.enter_context(tc.tile_pool(name="ptp", bufs=1))
        atp = ctx.enter_context(tc.tile_pool(name="atp", bufs=1))
        rp2 = ctx.enter_context(tc.tile_pool(name="rp2", bufs=2))
        osp = ctx.enter_context(tc.tile_pool(name="osp", bufs=4))
        ps = ctx.enter_context(tc.tile_pool(name="ps", bufs=1, space="PSUM"))

        bqk_sb = con.tile([P, 4], F32, tag="bqk")
        qkw_sb = con.tile([P, 4], F32, tag="qkw")

        # ---- constants (compute engines; no DMA) ----
        ones_m = con.tile([P, P], F16, tag="onesm")      # lhsT for sumsq
        nc.vector.memset(ones_m[:], 1.0)
        eps_sb = con.tile([P, 1], F32, tag="eps")
        nc.vector.memset(eps_sb[:], EPS)
        # ---- weight tiles (DMAs interleaved with x below) ----
        wv_sb = [wp.tile([P, CL], F16, tag=f"wv{k}", name=f"wv{k}") for k in range(kt)]
        wqk_sb = [wp.tile([P, 2 * CL], F16, tag=f"wqk{k}", name=f"wqk{k}") for k in range(kt)]
        wpr_sb = [wp.tile([P, C], F16, tag=f"wpr{t}", name=f"wpr{t}") for t in range(CL // P)]

        # rotating x tiles: 2 j-blocks in flight per k
        def x_tile(j, k):
            return xp.tile([P, nq], F16, tag=f"xt{k}", bufs=2, name=f"xt{k}_{j}")

        # persistent attention operands
        qkT = [qk.tile([P, n], F16, tag=f"qkT{m}", name=f"qkT{m}") for m in range(4)]
        v_aug = [vp.tile([P, HL, VW], F16, tag=f"va{i}", name=f"va{i}") for i in range(nb)]
        attnT = [atp.tile([P, n], F16, tag=f"at{t}", name=f"at{t}") for t in range(HL // HPB)]

        # ones columns of v_aug are written once here; the per-block drains
        # only touch the v columns (strided copy)
        ones_c = con.tile([P, HL, 1], F16, tag="onesc")
        nc.gpsimd.memset(ones_c[:], 1.0)
        for i in range(nb):
            nc.gpsimd.tensor_copy(v_aug[i][:, :, D:VW], ones_c[:])

        # startup: v weights + x(j=0) interleaved, consts slotted in after
        # the first pair, then qk/proj weights.
        xs = {}
        for k in range(kt):
            nc.sync.dma_start(wv_sb[k][:], wv_d[k * P:(k + 1) * P, :])
            t = x_tile(0, k)
            nc.sync.dma_start(t[:], xT_d[k * P:(k + 1) * P, 0:nq])
            xs[(0, k)] = t
        nc.sync.dma_start(bqk_sb[:], bqk_d[:])
        nc.sync.dma_start(qkw_sb[:], qkw_d[:])
        for k in range(kt):
            nc.sync.dma_start(wqk_sb[k][:], wqk_d[k * P:(k + 1) * P, :])
        for t in range(CL // P):
            nc.sync.dma_start(wpr_sb[t][:], wpr_d[t * P:(t + 1) * P, :])

        # ---- stage 1+2, pipelined per token block j ----
        for j in range(nj):
            js = slice(j * nq, (j + 1) * nq)
            if j + 1 < nj:
                for k in range(kt):
                    t = x_tile(j + 1, k)
                    nc.sync.dma_start(
                        t[:], xT_d[k * P:(k + 1) * P,
                                   (j + 1) * nq:(j + 2) * nq])
                    xs[(j + 1, k)] = t

            # v for this block's nq//P key blocks (b_v is folded into the
            # host-side output bias since attention rows sum to 1); the
            # strided drain leaves the ones columns intact
            for i in range(j * (nq // P), (j + 1) * (nq // P)):
                ioff = i * P - j * nq
                acc = ps.tile([P, HL, D], F32, tag="s2", bufs=2, name="vacc")
                for k in range(kt):
                    nc.tensor.matmul(
                        acc[:], xs[(j, k)][:, ioff:ioff + P], wv_sb[k][:],
                        start=(k == 0), stop=(k == kt - 1))
                nc.scalar.copy(v_aug[i][:, :, 0:D], acc[:])

            # qk projection + rmsnorm, k-heads (m=2,3) first: attention's
            # kT dependency clears while the q half of the last block
            # computes
            def emit_qk(m):
                # bias-add + PSUM drain on the scalar engine (Identity with
                # per-partition bias; identity lives in every act table set
                # so this never costs a table load)
                acc = ps.tile([P, nq], F32, tag="s2", bufs=2, name="qacc")
                for k in range(kt):
                    nc.tensor.matmul(
                        acc[:], wqk_sb[k][:, m * P:(m + 1) * P],
                        xs[(j, k)][:, :], start=(k == 0), stop=(k == kt - 1))
                nc.scalar.activation(qkT[m][:, js], acc[:], AF.Identity,
                                     bias=bqk_sb[:, m:m + 1])

            def emit_rms_for(jr):
                jrs = slice(jr * nq, (jr + 1) * nq)

                def emit_rms(m):
                    # both heads of the m-slice share one ssq tile: the
                    # h2=1 sumsq matmul is tile-positioned at partition 64
                    sq = sqp.tile([P, nq], F16, tag="sq", name="sq")
                    nc.scalar.square(sq[:], qkT[m][:, jrs])
                    ssq = ps.tile([P, nq], F32, tag="oas", bufs=4, name="ssq")
                    for h2 in range(HPB):
                        pr = slice(h2 * D, (h2 + 1) * D)
                        nc.tensor.matmul(ssq[pr, :], ones_m[pr, 0:D],
                                         sq[pr, :], start=True, stop=True)
                    rms = rp.tile([P, nq], F32, tag="rms", bufs=4, name="rms")
                    nc.scalar.activation(rms[:], ssq[:], AF.Sqrt,
                                         scale=1.0 / D, bias=eps_sb[:, 0:1])
                    rec = rp.tile([P, nq], F32, tag="rec", bufs=4, name="rec")
                    nc.vector.reciprocal_approx_fast(rec[:], rms[:])
                    nc.vector.scalar_tensor_tensor(
                        qkT[m][:, jrs], qkT[m][:, jrs], qkw_sb[:, m:m + 1],
                        rec[:], op0=mybir.AluOpType.mult,
                        op1=mybir.AluOpType.mult)

                return emit_rms

            if j < nj - 1:
                # interleave the deferred rms chains between the qk
                # projections so the ssq matmuls never park the PE behind
                # scalar-engine work that is queued after later qk drains
                emit_qk(2)
                emit_qk(3)
                if j > 0:
                    emit_rms_for(j - 1)(2)
                    emit_rms_for(j - 1)(3)
                emit_qk(0)
                emit_qk(1)
                if j > 0:
                    emit_rms_for(j - 1)(0)
                    emit_rms_for(j - 1)(1)
            else:
                # last block: norm the k heads before the q projections so
                # the attention's kT dependency clears under the q matmuls
                emit_qk(2)
                emit_qk(3)
                if j > 0:
                    emit_rms_for(j - 1)(2)
                    emit_rms_for(j - 1)(3)
                emit_rms_for(j)(2)
                emit_rms_for(j)(3)
                emit_qk(0)
                emit_qk(1)
                if j > 0:
                    emit_rms_for(j - 1)(0)
                    emit_rms_for(j - 1)(1)

        emit_rms_for(nj - 1)(0)
        emit_rms_for(nj - 1)(1)

        # ---- attention: S(cur) || AV(prev), then norm(prev2) + proj ----
        units = [(j, hp) for j in range(nj) for hp in range(HL // HPB)]
        BLK = min(4, nb)

        def emit_s(u, i):
            j, hp = u
            js = slice(j * nq, (j + 1) * nq)
            qm, km = hp, 2 + hp
            s2 = ps.tile([P, 2 * nq], F32, tag="s2", bufs=2, name="s2")
            for sub in range(HPB):
                pr = slice(sub * D, (sub + 1) * D)
                nc.tensor.matmul(
                    s2[:, sub * nq:(sub + 1) * nq],
                    qkT[km][pr, i * P:(i + 1) * P], qkT[qm][pr, js],
                    start=True, stop=True)
            pt = ptp.tile([P, 2 * nq], F16, tag="pt", bufs=20, name="pt")
            nc.scalar.activation(pt[:], s2[:], AF.Exp, scale=SCALE)
            return pt

        def emit_av(u, oas, pts, i):
            j, hp = u
            for sub in range(HPB):
                h = hp * HPB + sub
                nc.tensor.matmul(
                    oas[sub][0:VW, :], v_aug[i][:, h:h + 1, :],
                    pts[i][:, sub * nq:(sub + 1) * nq],
                    start=(i == 0), stop=(i == nb - 1))

        BCAST0 = [0] * 32

        def emit_norm(u, oas):
            # Softmax denominators ride in row 64 of each AV output.
            # stream_shuffle broadcasts partition 64 across two 32-row
            # quadrants (it shuffles within 32-partition windows, so two
            # ops), then reciprocal into SBUF and normalize into attnT.
            j, hp = u
            js = slice(j * nq, (j + 1) * nq)
            for sub in range(HPB):
                bcs = rp2.tile([D, nq], F32, tag=f"bcs{sub}", name="bcs")
                nc.vector.stream_shuffle(bcs[0:32, :], oas[sub][D:D + 32, :],
                                         BCAST0)
                nc.vector.stream_shuffle(bcs[32:64, :], oas[sub][D:D + 32, :],
                                         BCAST0)
                recn = rp2.tile([D, nq], F32, tag=f"recn{sub}", name="recn")
                nc.vector.reciprocal_approx_fast(recn[:], bcs[:])
                pr = slice(sub * D, (sub + 1) * D)
                nc.vector.tensor_mul(attnT[hp][pr, js], oas[sub][0:D, :],
                                     recn[:])

        def emit_proj_m(j, m):
            # b_proj (and the folded b_v term) are added on the host, so
            # the drain is a plain PSUM->fp16 copy
            js = slice(j * nq, (j + 1) * nq)
            acc = ps.tile([P, nq], F32, tag="s2", bufs=2, name="pacc")
            for t in range(CL // P):
                nc.tensor.matmul(
                    acc[:], wpr_sb[t][:, m * P:(m + 1) * P],
                    attnT[t][:, js], start=(t == 0), stop=(t == CL // P - 1))
            ost = osp.tile([P, nq], F16, tag="ost", name="ost")
            nc.vector.tensor_copy(ost[:], acc[:])
            nc.sync.dma_start(outT_d[m * P:(m + 1) * P, js], ost[:])

        nchunks = (nb + BLK - 1) // BLK
        mpc = (C // P) // nchunks      # proj m-steps per chunk
        prev = None    # (unit, oas, pts)
        prev2 = None
        for idx in range(len(units) + 2):
            cur = units[idx] if idx < len(units) else None
            # norm of prev2 first: its reads gate the slot reuse of this
            # iteration's AV writes, so it must precede them in scheduler
            # priority (emitting it later deadlocks the in-order PE queue)
            proj_j = None
            if prev2 is not None:
                emit_norm(prev2[0], prev2[1])
                j2, hp2 = prev2[0]
                if hp2 == HL // HPB - 1:
                    proj_j = j2
            oas_prev = None
            if prev is not None:
                oas_prev = [ps.tile([P, nq], F32, tag="oas", bufs=4,
                                    name=f"oa{s_}") for s_ in range(HPB)]
            pts = {}
            for ib in range(nchunks):
                blk = range(ib * BLK, min((ib + 1) * BLK, nb))
                if cur is not None:
                    for i in blk:
                        pts[i] = emit_s(cur, i)
                if prev is not None:
                    for i in blk:
                        emit_av(prev[0], oas_prev, prev[2], i)
                # proj of the finished block rides along, a couple of
                # m-slices per chunk, so its PSUM slot rotation (and the
                # drain DVE work) spreads across the unit instead of
                # stalling the PE in one burst
                if proj_j is not None:
                    for m in range(ib * mpc, (ib + 1) * mpc):
                        emit_proj_m(proj_j, m)
            if proj_j is not None:
                for m in range(nchunks * mpc, C // P):
                    emit_proj_m(proj_j, m)
            prev2 = (prev[0], oas_prev) if prev is not None else None
            prev = (cur, None, pts) if cur is not None else None

    nc.compile()
    return nc


_NC_CACHE = {}


def _get_nc(n=N, nq=NQ):
    key = (n, nq)
    if key not in _NC_CACHE:
        _NC_CACHE[key] = build(n, nq)
    return _NC_CACHE[key]


def make_in_maps(x, w_qkv, b_qkv, q_w, k_w, w_proj, b_proj):
    """Shard full inputs into per-core in_maps (host side)."""
    in_maps = []
    for cid in range(NCORES):
        b, g = cid // GROUPS, cid % GROUPS
        c0 = g * CL
        xT = np.ascontiguousarray(x[b].T)
        w_qk = np.ascontiguousarray(
            np.concatenate([w_qkv[:, c0:c0 + CL],
                            w_qkv[:, C + c0:C + c0 + CL]], axis=1))
        w_v = np.ascontiguousarray(w_qkv[:, 2 * C + c0:2 * C + c0 + CL])
        w_pr = np.ascontiguousarray(w_proj[c0:c0 + CL, :])
        b_qk = np.stack([b_qkv[c0 + m * P:c0 + (m + 1) * P] for m in range(2)]
                        + [b_qkv[C + c0 + m * P:C + c0 + (m + 1) * P]
                           for m in range(2)], axis=1)
        qkw = np.stack([np.tile(q_w, HPB), np.tile(q_w, HPB),
                        np.tile(k_w, HPB), np.tile(k_w, HPB)], axis=1)
        in_maps.append({
            "xT": xT.astype(np.float16),
            "w_qk": w_qk.astype(np.float16),
            "w_v": w_v.astype(np.float16),
            "w_pr": w_pr.astype(np.float16),
            "b_qk": np.ascontiguousarray(b_qk).astype(np.float32),
            "qkw": np.ascontiguousarray(qkw).astype(np.float32),
        })
    return in_maps


def kernel(x, w_qkv, b_qkv, q_w, k_w, w_proj, b_proj, _trace=False):
    x = np.asarray(x)
    n = x.shape[1]
    nc = _get_nc(n, NQ if n % NQ == 0 else P)
    in_maps = make_in_maps(np.asarray(x, np.float32), np.asarray(w_qkv, np.float32),
                           np.asarray(b_qkv, np.float32), np.asarray(q_w, np.float32),
                           np.asarray(k_w, np.float32), np.asarray(w_proj, np.float32),
                           np.asarray(b_proj, np.float32))
    res = run_bass_kernel_spmd(nc, in_maps, core_ids=list(range(NCORES)),
                               trace=_trace)
    # TP unshard: sum the 4 head-group fp16 partials per batch, transpose,
    # and add the host-folded bias (b_proj plus b_v @ w_proj: attention
    # rows sum to 1, so the v bias is additive on attn_out)
    bias = (np.asarray(b_proj, np.float32)
            + np.asarray(b_qkv[2 * C:3 * C], np.float32)
            @ np.asarray(w_proj, np.float32))
    out = np.stack([
        sum(res.results[b * GROUPS + g]["outT"].astype(np.float32)
            for g in range(GROUPS)).T + bias
        for b in range(B)
    ]).astype(np.float32)
    if _trace:
        return out, res
    return out
